# revision 16
# baseline (speedup 1.0000x reference)
"""Trainium2 Bass kernel for CommunityPassing (segment mean + gather).

Algorithm (8 NeuronCores, data-parallel over nodes):
  host: shard x/community over 8 cores along the node axis; within each
        shard, stably sort node indices by community id and pack them into
        128-row tiles grouped by community "chunk" (128 communities per
        chunk, 8 chunks for 1000 communities). Pad each (core, chunk)
        block to a shared tile count so all cores run one SPMD program.
        Precompute the fp8 one-hot B^T [comm, sorted_node] (exact for
        0/1) for phase 2, per-tile local ids and 1/count.
  dev:  phase 1 - stream sorted x tiles (bf16); build per-tile one-hot
        B[node, comm] with a DVE is_equal; matmul lhsT=B @ x_tile
        accumulating into a PSUM tile per community chunk.
        Partial sums are pre-scaled by 1/count and cast to bf16 BEFORE
        the collective, so each 2-chunk bf16 AllReduce directly produces
        the mean table slice with zero post-collective compute; the AR
        issue and AR-result DMA live on the GpSimd queue so no streaming
        engine ever blocks on a collective.
        phase 2 (software-pipelined 4 chunks behind phase 1) - per
        (chunk, feature-half), matmul lhsT=mean[c, f128] (stationary)
        with wide rhs=B^T[c, n512] fp8 slices -> out^T [f128, n] in
        PSUM ([128, 1024] double-bank tiles, two matmuls per tile); the
        Scalar engine copies PSUM -> bf16 staging and issues the out^T
        DMAs from its own queue, keeping the SP queue purely input-side.
  host: upcast bf16 -> fp32, transpose, unsort, concatenate the 8
        output shards.
"""

import os
import sys

import numpy as np

for _p in ("/opt/trn_rl_repo", "/opt/pypackages"):
    if _p not in sys.path and os.path.isdir(_p):
        sys.path.append(_p)

# Problem constants (hardcoded per the task contract).
N_FULL = 500000
F = 256
NUM_COMMS = 1000
EPS = 1e-12
M = 8               # cores
P = 128             # partitions
NC_CHUNKS = 8       # community chunks of 128 (8*128 = 1024 >= 1000)
# All-reduce chunk groups: small first group -> the phase-2 pipeline can
# start early; small last group -> short drain tail behind phase 1.
AR_GROUPS = ((0,), (1, 2), (3, 4), (5, 6), (7,))
LAG = 4             # phase-2 chunk pipeline lag behind phase 1
XB = 32             # phase-1 x tiles per streaming DMA (32*512B = 16KB lines)
NB = 512            # phase-2 nodes per matmul (psum bank = 512 fp32)
KB2 = 2 * NB        # phase-2 psum tile columns (2 banks, 2 matmuls, 1 copy)
OSTG = 4096         # phase-2 out staging columns per DMA (8KB lines)

# Stash of the most recent run's BassKernelResults (for test harnesses).
LAST_RESULTS = None


def _host_prep(x, community):
    """Build per-core device inputs. Returns (in_maps, plan)."""
    import ml_dtypes

    x = np.ascontiguousarray(np.asarray(x, dtype=np.float32))
    community = np.asarray(community).astype(np.int64)
    n = x.shape[0]
    assert n % M == 0
    nl = n // M

    comm_sh = community.reshape(M, nl)
    perms = np.argsort(comm_sh, axis=1, kind="stable")
    comm_sorted = np.take_along_axis(comm_sh, perms, axis=1)

    # per (core, chunk) node counts
    chunk_ids = comm_sorted >> 7  # // 128
    cnts = np.zeros((M, NC_CHUNKS), dtype=np.int64)
    for m in range(M):
        bc = np.bincount(chunk_ids[m], minlength=NC_CHUNKS)
        cnts[m] = bc[:NC_CHUNKS]
    t_k = np.maximum(1, -(-cnts.max(axis=0) // P))  # ceil, shared by all cores
    t_total = int(t_k.sum())
    tile_off = np.concatenate([[0], np.cumsum(t_k)])  # tile index base per chunk

    # counts -> 1/max(cnt, eps), [p, k] layout (community id = k*128 + p)
    cnt_full = np.bincount(community, minlength=NUM_COMMS).astype(np.float32)
    inv_pad = np.zeros((NC_CHUNKS * P,), np.float32)
    inv_pad[:NUM_COMMS] = 1.0 / np.maximum(cnt_full, np.float32(EPS))
    invc = np.ascontiguousarray(inv_pad.reshape(NC_CHUNKS, P).T)  # [128, 8]

    iota = np.ascontiguousarray(
        np.tile(np.arange(P, dtype=ml_dtypes.bfloat16), (P, 1))
    )  # [128, 128] bf16, each row 0..127

    arange_p = np.arange(P, dtype=np.int32)
    in_maps = []
    origs = []
    for m in range(M):
        x_m = x[m * nl : (m + 1) * nl]
        xs = np.zeros((t_total * P, F), dtype=np.float32)
        locid = np.full((t_total * P,), -1, dtype=np.int32)
        orig = np.full((t_total * P,), -1, dtype=np.int64)
        start = 0
        for k in range(NC_CHUNKS):
            c = int(cnts[m, k])
            row = int(tile_off[k]) * P
            sel = perms[m, start : start + c]
            xs[row : row + c] = x_m[sel]
            orig[row : row + c] = sel
            locid[row : row + c] = (comm_sorted[m, start : start + c] - k * P).astype(
                np.int32
            )
            start += c
        origs.append(orig)

        # xs: [128, T, 256] bf16 -- xs_dev[p, t, f] = x_sorted[t*128+p, f]
        xs_dev = np.ascontiguousarray(
            xs.reshape(t_total, P, F).transpose(1, 0, 2).astype(ml_dtypes.bfloat16)
        )
        # locid: [128, T] fp32 (per-partition scalar for phase-1 is_equal)
        locid_t = np.ascontiguousarray(locid.reshape(t_total, P).T.astype(np.float32))
        # BT: [128, NT] fp8e4 -- BT[c, n] = 1 iff locid[n] == c (sorted order)
        bt = (locid[None, :] == arange_p[:, None]).astype(ml_dtypes.float8_e4m3)
        in_maps.append(
            {
                "xs": xs_dev,
                "locid": locid_t,
                "bt": np.ascontiguousarray(bt),
                "iota": iota,
                "invc": invc,
            }
        )

    plan = {
        "nl": nl,
        "t_k": [int(v) for v in t_k],
        "t_total": t_total,
        "tile_off": [int(v) for v in tile_off],
        "origs": origs,
    }
    return in_maps, plan


def _build_program(plan, use_collective=True):
    from concourse import bacc, mybir, tile

    t_total = plan["t_total"]
    tile_off = plan["tile_off"]
    nt = t_total * P

    dt = mybir.dt
    nc = bacc.Bacc("TRN2", target_bir_lowering=False, debug=False, num_devices=M)

    xs = nc.dram_tensor("xs", [P, t_total, F], dt.bfloat16, kind="ExternalInput")
    locid = nc.dram_tensor("locid", [P, t_total], dt.float32, kind="ExternalInput")
    btd = nc.dram_tensor("bt", [P, nt], dt.float8e4, kind="ExternalInput")
    iota = nc.dram_tensor("iota", [P, P], dt.bfloat16, kind="ExternalInput")
    invc = nc.dram_tensor("invc", [P, NC_CHUNKS], dt.float32, kind="ExternalInput")
    out = nc.dram_tensor("out", [2 * P, nt], dt.bfloat16, kind="ExternalOutput")

    group_of_chunk = {}
    for g, ch in enumerate(AR_GROUPS):
        for k in ch:
            group_of_chunk[k] = g

    with tile.TileContext(nc) as tc:
        with (
            tc.tile_pool(name="const", bufs=1) as constp,
            tc.tile_pool(name="xsp", bufs=3) as xsp,
            tc.tile_pool(name="btp", bufs=3) as btp,
            tc.tile_pool(name="bp", bufs=8) as bp,
            tc.tile_pool(name="acc", bufs=1) as accp,
            tc.tile_pool(name="outp", bufs=4) as outp,
            tc.tile_pool(name="ps1", bufs=2, space="PSUM") as ps1,
            tc.tile_pool(name="ps2", bufs=3, space="PSUM") as ps2,
            tc.tile_pool(name="dram", bufs=1, space="DRAM") as dramp,
        ):
            iota_t = constp.tile([P, P], dt.bfloat16)
            nc.sync.dma_start(out=iota_t[:], in_=iota.ap())
            locid_t = constp.tile([P, t_total], dt.float32)
            nc.sync.dma_start(out=locid_t[:], in_=locid.ap())
            invc_t = constp.tile([P, NC_CHUNKS], dt.float32)
            nc.sync.dma_start(out=invc_t[:], in_=invc.ap())

            comm_sum = accp.tile([P, NC_CHUNKS * F], dt.float32)
            arsrc = accp.tile([P, NC_CHUNKS * F], dt.bfloat16)  # pre-scaled
            mean_bf = accp.tile([P, NC_CHUNKS * F], dt.bfloat16)
            ar_in = [
                dramp.tile([P, len(ch) * F], dt.bfloat16, name=f"ar_in{g}")
                for g, ch in enumerate(AR_GROUPS)
            ]
            ar_out = [
                dramp.tile([P, len(ch) * F], dt.bfloat16, name=f"ar_out{g}")
                for g, ch in enumerate(AR_GROUPS)
            ]

            def emit_phase1_chunk(k, state):
                """Phase-1 tiles for chunk k + AR-group chain when ready."""
                for t in range(tile_off[k], tile_off[k + 1]):
                    if t % XB == 0:
                        w = min(XB, t_total - t)
                        state["xsb"] = xsp.tile(
                            [P, XB * F], dt.bfloat16, tag="xsb", name="xsb"
                        )
                        nc.sync.dma_start(
                            out=state["xsb"][:, : w * F].rearrange(
                                "p (b f) -> p b f", b=w
                            ),
                            in_=xs.ap()[:, t : t + w, :],
                        )
                    first = t == tile_off[k]
                    last = t == tile_off[k + 1] - 1
                    if first:
                        state["psum"] = ps1.tile([P, F], dt.float32, name="psum1")
                    b_t = bp.tile([P, P], dt.bfloat16, tag="b", name="b_t")
                    nc.vector.tensor_scalar(
                        b_t[:],
                        iota_t[:],
                        locid_t[:, t : t + 1],
                        None,
                        mybir.AluOpType.is_equal,
                    )
                    j = t % XB
                    nc.tensor.matmul(
                        state["psum"][:],
                        lhsT=b_t[:],
                        rhs=state["xsb"][:, j * F : (j + 1) * F],
                        start=first,
                        stop=last,
                    )
                    if last:
                        nc.scalar.copy(
                            out=comm_sum[:, k * F : (k + 1) * F],
                            in_=state["psum"][:],
                        )
                # pre-scale by 1/count -> bf16 AR payload (off critical path)
                nc.vector.tensor_scalar(
                    arsrc[:, k * F : (k + 1) * F],
                    comm_sum[:, k * F : (k + 1) * F],
                    invc_t[:, k : k + 1],
                    None,
                    mybir.AluOpType.mult,
                )
                g = group_of_chunk[k]
                if k == AR_GROUPS[g][-1]:
                    # Collective chain on SP(ar_in write) + GpSimd only.
                    lo = AR_GROUPS[g][0] * F
                    arw = len(AR_GROUPS[g]) * F
                    nc.sync.dma_start(out=ar_in[g], in_=arsrc[:, lo : lo + arw])
                    if use_collective:
                        nc.gpsimd.collective_compute(
                            "AllReduce",
                            mybir.AluOpType.add,
                            replica_groups=[list(range(M))],
                            ins=[ar_in[g].opt()],
                            outs=[ar_out[g].opt()],
                        )
                        nc.gpsimd.dma_start(
                            out=mean_bf[:, lo : lo + arw], in_=ar_out[g]
                        )
                    else:
                        nc.gpsimd.dma_start(
                            out=mean_bf[:, lo : lo + arw], in_=ar_in[g]
                        )

            ncopy = [0]

            def emit_phase2_chunk(k):
                """outT[f, n] = mean_chunk[c, f].T @ BT[c, n] for chunk k."""
                lo = tile_off[k] * P
                hi = tile_off[k + 1] * P
                btb = btp.tile([P, hi - lo], dt.float8e4, tag="btb", name="btb")
                nc.sync.dma_start(out=btb[:], in_=btd.ap()[:, lo:hi])
                for h in range(2):
                    stg = None
                    fill = 0
                    base = lo
                    pso = None
                    pfill = 0
                    for n0 in range(lo, hi, NB):
                        w = min(NB, hi - n0)
                        if stg is None:
                            stg = outp.tile(
                                [P, OSTG], dt.bfloat16, tag="stg", name="stg"
                            )
                            base = n0
                            fill = 0
                        if pfill == 0:
                            pso = ps2.tile([P, KB2], dt.float32, tag="pso", name="pso")
                        nc.tensor.matmul(
                            pso[:, pfill : pfill + w],
                            lhsT=mean_bf[:, k * F + h * P : k * F + (h + 1) * P],
                            rhs=btb[:, n0 - lo : n0 - lo + w],
                            start=True,
                            stop=True,
                        )
                        pfill += w
                        if pfill == KB2 or n0 + w == hi:
                            dst = stg[:, fill : fill + pfill]
                            if ncopy[0] % 2 == 0:
                                nc.scalar.copy(out=dst, in_=pso[:, :pfill])
                            else:
                                nc.vector.tensor_copy(out=dst, in_=pso[:, :pfill])
                            ncopy[0] += 1
                            fill += pfill
                            pfill = 0
                        if fill == OSTG or n0 + w == hi:
                            nc.scalar.dma_start(
                                out=out.ap()[
                                    h * P : (h + 1) * P, base : base + fill
                                ],
                                in_=stg[:, :fill],
                            )
                            stg = None

            # Software pipeline: phase 2 trails phase 1 by LAG chunks so its
            # mean slice (2-chunk AllReduce) is ready before the PE reaches
            # the corresponding phase-2 matmuls.
            state = {}
            for i in range(NC_CHUNKS + LAG):
                if i < NC_CHUNKS:
                    emit_phase1_chunk(i, state)
                if i >= LAG:
                    emit_phase2_chunk(i - LAG)

    nc.compile()
    return nc


def kernel(x, community):
    global LAST_RESULTS
    from concourse.bass_utils import run_bass_kernel_spmd

    in_maps, plan = _host_prep(x, community)
    nc = _build_program(plan)
    res = run_bass_kernel_spmd(nc, in_maps, core_ids=list(range(M)))
    LAST_RESULTS = res
    nl = plan["nl"]
    outs = []
    for m in range(M):
        od = np.asarray(res.results[m]["out"])  # [256, NT] bf16, outT
        out_sorted = od.T.astype(np.float32)  # [NT, 256]
        orig = plan["origs"][m]
        valid = orig >= 0
        out_m = np.empty((nl, F), dtype=np.float32)
        out_m[orig[valid]] = out_sorted[valid]
        outs.append(out_m)
    return np.concatenate(outs, axis=0)
